# revision 16
# baseline (speedup 1.0000x reference)
"""Trainium2 Bass kernel for the LTPE block:

    out_j = conv3x3(x, kernel_j)   (8 kernels: [-1 at neighbor j, +1 at center])
    out   = sum_j ((out_j + 1) * 0.5) * (2**j / 255)
    out   = InstanceNorm2d(out)    (per-sample over H,W, eps=1e-5, no affine)

Math: sum_j 2**j/255 == 1, so
    out = 0.5*(x - conv) + 0.5,  conv = sum_j (2**j/255) * shift_j(x)
InstanceNorm is invariant to the affine: with z = 255*x - sum_j 2**j*shift_j(x)
    result = (z - mean(z)) / sqrt(var(z) + 260100e-5)
z is computed as a 3x3 stencil via banded [128,128] matmuls (one per column
shift) on the tensor engine, with bf16 hi/lo splitting of x for ~fp32
accuracy (hi = bf16(x) on ScalarE, lo = x - hi on GpSimd, both streamed into
the same PSUM accumulation).  Pure data parallel: 4 samples/core, 8 cores.

Row tiling: tile t computes output rows [126t, 126t+126) (last tile: 16 rows)
from input rows [126t-1, 126t+127).  Output row 126t+n sits at partition n;
the vertical taps form a banded matrix with band (0,1,2) for t>0 and
(-1,0,1) for t=0 (zero-pad rows handled by band clipping / K=17 on the tail).

Samples are software-pipelined at tile granularity: the finalize chain of
sample s-1 (stats aggregation, normalize, store) is emitted in small chunks
between the tile emissions of sample s, so the static per-engine instruction
streams never head-of-line block across the sample boundary.
"""

import numpy as np
import ml_dtypes

import concourse.bass as bass
import concourse.tile as tile
from concourse import mybir
from concourse.bacc import Bacc
from concourse.bass_utils import run_bass_kernel_spmd

N_CORES = 8
B_PER_CORE = 4
H = W = 1024
TO = 126           # output rows per tile (input rows = TO + 2 halo)
NT = 9             # 8 full tiles + 16-row tail
TAIL = H - 8 * TO  # 16
EPS_P = 260100e-5  # 255^2 * 4 * 1e-5 : the InstanceNorm eps after rescaling

# neighbor offsets (dy, dx) for weights 2**j
_OFFSETS = [(0, -1), (1, -1), (1, 0), (1, 1), (0, 1), (-1, 1), (-1, 0), (-1, -1)]

F32 = mybir.dt.float32
BF16 = mybir.dt.bfloat16
ALU = mybir.AluOpType
AF = mybir.ActivationFunctionType


def _build_host_weights():
    """Banded matrices V[dx][k, n]: coefficient of input partition k for
    output partition n, for column shift dx.  Band "a" (t=0): input row at
    partition k is row k, out row n -> taps k=n+dy.  Band "b" (t>0): input
    row at partition k is 126t-1+k, out row 126t+n -> taps k=n+1+dy."""
    out = {}
    for name, shift in (("a", 0), ("b", 1)):
        V = {dx: np.zeros((128, 128), np.float32) for dx in (-1, 0, 1)}
        for n in range(128):
            k = n + shift
            if k < 128:
                V[0][k, n] = 255.0  # center tap (+255 x)
        for j, (dy, dx) in enumerate(_OFFSETS):
            for n in range(128):
                k = n + shift + dy
                if 0 <= k < 128:
                    V[dx][k, n] += -float(2 ** j)
        for dx, tag in ((-1, "l"), (0, "c"), (1, "r")):
            out[f"v{tag}{name}"] = np.ascontiguousarray(
                V[dx].astype(ml_dtypes.bfloat16)
            )

    # cross-partition count weights: row k weighted n_k / (H*W); all 128
    # output columns identical -> the matmul broadcasts the totals.
    counts = np.zeros((128,), np.float64)
    for t in range(NT):
        n_out = TO if t < 8 else TAIL
        counts[0:n_out] += W
    wcnt = np.tile((counts / float(H * W)).astype(np.float32)[:, None], (1, 128))
    out["wcnt"] = np.ascontiguousarray(wcnt, dtype=np.float32)
    return out


def _mm_plan(h, lo_on):
    """(weight, use_lo, in_c0, in_c1, out_c0, out_c1) per matmul for one PSUM
    half: column shifts realized by sliding the moving operand's columns."""
    c0 = 512 * h
    plan = []
    for vname in ("vc", "vl", "vr"):
        for src_lo in [False] + ([True] if vname in lo_on else []):
            if vname == "vc":
                plan.append((vname, src_lo, c0, c0 + 512, 0, 512))
            elif vname == "vl":
                if h == 0:
                    plan.append((vname, src_lo, 0, 511, 1, 512))
                else:
                    plan.append((vname, src_lo, 511, 1023, 0, 512))
            else:  # vr
                if h == 0:
                    plan.append((vname, src_lo, 1, 513, 0, 512))
                else:
                    plan.append((vname, src_lo, 513, 1024, 0, 511))
    return plan


def build_nc(mode="hilo", lo_passes=("vl", "vc", "vr")):
    lo_on = lo_passes if mode == "hilo" else ()
    use_lo = len(lo_on) > 0
    nc = Bacc()
    x_in = nc.declare_dram_parameter("x", [B_PER_CORE, 1, H, W], F32, isOutput=False)
    out_ext = nc.declare_dram_parameter("out", [B_PER_CORE, 1, H, W], F32, isOutput=True)
    w_names = ["vla", "vca", "vra", "vlb", "vcb", "vrb"]
    w_dram = {
        n: nc.declare_dram_parameter(n, [128, 128], BF16, isOutput=False)
        for n in w_names
    }
    wcnt_d = nc.declare_dram_parameter("wcnt", [128, 128], F32, isOutput=False)

    with tile.TileContext(nc) as tc:
        with (
            tc.tile_pool(name="singles", bufs=1) as singles,
            tc.tile_pool(name="xp", bufs=8) as xp,
            tc.tile_pool(name="hp", bufs=6) as hp,
            tc.tile_pool(name="lp", bufs=6) as lp,
            tc.tile_pool(name="zp", bufs=2) as zp,
            tc.tile_pool(name="stat", bufs=2) as stat,
            tc.tile_pool(name="sm", bufs=4) as sm,
            tc.tile_pool(name="psp", bufs=3, space="PSUM") as psp,
            tc.tile_pool(name="pss", bufs=1, space="PSUM") as pss,
        ):
            sb_v = {}
            for n in w_names:
                t_ = singles.tile([128, 128], BF16, tag=n)
                nc.sync.dma_start(out=t_, in_=w_dram[n][:, :])
                sb_v[n] = t_
            sb_wcnt = singles.tile([128, 128], F32, tag="wcnt")
            nc.sync.dma_start(out=sb_wcnt, in_=wcnt_d[:, :])
            sb_eps = singles.tile([128, 1], F32, tag="eps")
            nc.vector.memset(sb_eps, EPS_P)

            def emit_tile(s, t, z_big, stats):
                if t == 0:
                    nc.gpsimd.memset(stats[:], 0.0)
                n_out = TO if t < 8 else TAIL
                in_a = max(TO * t - 1, 0)       # first input row loaded
                in_b = min(TO * t + TO + 1, H)  # one past last input row
                rows = in_b - in_a              # 127/128, or 17 on the tail
                K = rows                        # contraction depth
                band = "a" if t == 0 else "b"

                xt = xp.tile([128, W], F32, tag="xt")
                nc.sync.dma_start(out=xt[0:rows, :], in_=x_in[s, 0, in_a:in_b, :])
                hi = hp.tile([128, W], BF16, tag="hi")
                nc.scalar.copy(out=hi[0:rows, :], in_=xt[0:rows, :])
                lo = None
                if use_lo:
                    lo = lp.tile([128, W], BF16, tag="lo")
                    eng = nc.gpsimd if t % 2 == 0 else nc.vector
                    eng.tensor_sub(
                        lo[0:rows, :], xt[0:rows, :], hi[0:rows, :]
                    )

                ps = psp.tile([128, 2, 512], F32, tag="ps")
                for h in (0, 1):
                    plan = _mm_plan(h, lo_on)
                    for i, (vname, src_lo, a, b, oa, ob) in enumerate(plan):
                        src = lo if src_lo else hi
                        nc.tensor.matmul(
                            ps[:, h, oa:ob],
                            lhsT=sb_v[vname + band][0:K, :],
                            rhs=src[0:K, a:b],
                            start=(i == 0),
                            stop=(i == len(plan) - 1),
                        )

                nc.scalar.copy(
                    out=z_big[0:n_out, t, :].rearrange("p (g f) -> p g f", f=512),
                    in_=ps[0:n_out, :, :],
                )
                for g in (0, 1):
                    nc.vector.bn_stats(
                        out=stats[0:n_out, t, g, :],
                        in_=z_big[0:n_out, t, 512 * g:512 * (g + 1)],
                    )

            def finalize_chunks(s, z_big, stats):
                box = {}

                def c1():
                    mv = box["mv"] = sm.tile([128, 2], F32, tag="mv", name="mv")
                    nc.vector.memset(mv, 0.0)
                    nc.vector.bn_aggr(out=mv[0:TO, :], in_=stats[0:TO, :, :, :])
                    msq = sm.tile([128, 1], F32, tag="msq")
                    nc.vector.tensor_mul(msq, mv[:, 0:1], mv[:, 0:1])
                    nc.vector.tensor_add(mv[:, 1:2], mv[:, 1:2], msq)  # E2

                def c2():
                    tot_ps = pss.tile([128, 2], F32, tag="totps")
                    nc.tensor.matmul(
                        tot_ps[:, :], lhsT=sb_wcnt[:, :], rhs=box["mv"][:, :],
                        start=True, stop=True,
                    )
                    tot = box["tot"] = sm.tile([128, 2], F32, tag="tot", name="tot")
                    nc.scalar.copy(out=tot, in_=tot_ps)

                def c3():
                    tot = box["tot"]
                    m2 = sm.tile([128, 1], F32, tag="m2")
                    nc.vector.tensor_mul(m2, tot[:, 0:1], tot[:, 0:1])
                    var = sm.tile([128, 1], F32, tag="var")
                    nc.vector.tensor_sub(var, tot[:, 1:2], m2)
                    sd = box["sd"] = sm.tile([128, 1], F32, tag="sd", name="sd")
                    nc.scalar.activation(
                        out=sd, in_=var, func=AF.Sqrt, bias=sb_eps, scale=1.0
                    )

                def c4():
                    inv = box["inv"] = sm.tile([128, 1], F32, tag="inv", name="inv")
                    nc.vector.reciprocal(inv, box["sd"])
                    nbias = box["nb"] = sm.tile([128, 1], F32, tag="nb", name="nb")
                    nc.vector.tensor_scalar(
                        out=nbias, in0=inv, scalar1=box["tot"][:, 0:1],
                        scalar2=-1.0, op0=ALU.mult, op1=ALU.mult,
                    )

                def norm_store(t0, t1):
                    def c():
                        nc.vector.tensor_scalar(
                            out=z_big[0:TO, t0:t1, :], in0=z_big[0:TO, t0:t1, :],
                            scalar1=box["inv"][0:TO, :],
                            scalar2=box["nb"][0:TO, :],
                            op0=ALU.mult, op1=ALU.add,
                        )
                        # output row 126t+n <-> (n, t) of z_big
                        nc.scalar.dma_start(
                            out=out_ext[s, 0, TO * t0:TO * t1, :].rearrange(
                                "(t n) w -> n t w", n=TO
                            ),
                            in_=z_big[0:TO, t0:t1, :],
                        )
                    return c

                def c_tail():
                    nc.vector.tensor_scalar(
                        out=z_big[0:TAIL, 8, :], in0=z_big[0:TAIL, 8, :],
                        scalar1=box["inv"][0:TAIL, :],
                        scalar2=box["nb"][0:TAIL, :],
                        op0=ALU.mult, op1=ALU.add,
                    )
                    nc.scalar.dma_start(
                        out=out_ext[s, 0, 8 * TO:H, :], in_=z_big[0:TAIL, 8, :]
                    )

                return [c1, c2, c3, c4,
                        norm_store(0, 4), norm_store(4, 8), c_tail]

            pending = []
            for s in range(B_PER_CORE):
                z_big = zp.tile([128, NT, W], F32, tag="z")
                stats = stat.tile([128, NT, 2, 6], F32, tag="stats")
                for t in range(NT):
                    emit_tile(s, t, z_big, stats)
                    if pending:
                        pending.pop(0)()
                while pending:
                    pending.pop(0)()
                pending = finalize_chunks(s, z_big, stats)
            while pending:
                pending.pop(0)()
    nc.finalize()
    return nc


_NC_CACHE = {}


def _get_nc(mode, lo_passes):
    key = (mode, tuple(lo_passes))
    if key not in _NC_CACHE:
        _NC_CACHE[key] = build_nc(mode, lo_passes)
    return _NC_CACHE[key]


def run(x, trace=False, mode="hilo", lo_passes=("vl", "vc", "vr"), tmpdir=None):
    x = np.ascontiguousarray(np.asarray(x), dtype=np.float32)
    assert x.shape == (N_CORES * B_PER_CORE, 1, H, W), x.shape
    weights = _build_host_weights()
    in_maps = []
    for c in range(N_CORES):
        m = {"x": x[c * B_PER_CORE:(c + 1) * B_PER_CORE]}
        m.update(weights)
        in_maps.append(m)
    nc = _get_nc(mode, lo_passes)
    res = run_bass_kernel_spmd(
        nc, in_maps, list(range(N_CORES)), trace=trace, tmpdir=tmpdir
    )
    out = np.concatenate([res.results[c]["out"] for c in range(N_CORES)], axis=0)
    return out, res


def kernel(x):
    out, _ = run(x, trace=False)
    return out


# revision 18
# speedup vs baseline: 1.0206x; 1.0206x over previous
"""Trainium2 Bass kernel for the LTPE block:

    out_j = conv3x3(x, kernel_j)   (8 kernels: [-1 at neighbor j, +1 at center])
    out   = sum_j ((out_j + 1) * 0.5) * (2**j / 255)
    out   = InstanceNorm2d(out)    (per-sample over H,W, eps=1e-5, no affine)

Math: sum_j 2**j/255 == 1, so
    out = 0.5*(x - conv) + 0.5,  conv = sum_j (2**j/255) * shift_j(x)
InstanceNorm is invariant to the affine: with z = 255*x - sum_j 2**j*shift_j(x)
    result = (z - mean(z)) / sqrt(var(z) + 260100e-5)
z is computed as a 3x3 stencil via banded [128,128] matmuls (one per column
shift) on the tensor engine, with bf16 hi/lo splitting of x for ~fp32
accuracy (hi = bf16(x) on ScalarE, lo = x - hi on GpSimd, both streamed into
the same PSUM accumulation).  Pure data parallel: 4 samples/core, 8 cores.

Row tiling: tile t computes output rows [126t, 126t+126) (last tile: 16 rows)
from input rows [126t-1, 126t+127).  Output row 126t+n sits at partition n;
the vertical taps form a banded matrix with band (0,1,2) for t>0 and
(-1,0,1) for t=0 (zero-pad rows handled by band clipping / K=17 on the tail).

Samples are software-pipelined at tile granularity: the finalize chain of
sample s-1 (stats aggregation, normalize, store) is emitted in small chunks
between the tile emissions of sample s, so the static per-engine instruction
streams never head-of-line block across the sample boundary.
"""

import numpy as np
import ml_dtypes

import concourse.bass as bass
import concourse.tile as tile
from concourse import mybir
from concourse.bacc import Bacc
from concourse.bass_utils import run_bass_kernel_spmd

N_CORES = 8
B_PER_CORE = 4
H = W = 1024
TO = 126           # output rows per tile (input rows = TO + 2 halo)
NT = 9             # 8 full tiles + 16-row tail
TAIL = H - 8 * TO  # 16
EPS_P = 260100e-5  # 255^2 * 4 * 1e-5 : the InstanceNorm eps after rescaling

# neighbor offsets (dy, dx) for weights 2**j
_OFFSETS = [(0, -1), (1, -1), (1, 0), (1, 1), (0, 1), (-1, 1), (-1, 0), (-1, -1)]

F32 = mybir.dt.float32
BF16 = mybir.dt.bfloat16
ALU = mybir.AluOpType
AF = mybir.ActivationFunctionType


def _build_host_weights():
    """Banded matrices V[dx][k, n]: coefficient of input partition k for
    output partition n, for column shift dx.  Band "a" (t=0): input row at
    partition k is row k, out row n -> taps k=n+dy.  Band "b" (t>0): input
    row at partition k is 126t-1+k, out row 126t+n -> taps k=n+1+dy."""
    out = {}
    for name, shift in (("a", 0), ("b", 1)):
        V = {dx: np.zeros((128, 128), np.float32) for dx in (-1, 0, 1)}
        for n in range(128):
            k = n + shift
            if k < 128:
                V[0][k, n] = 255.0  # center tap (+255 x)
        for j, (dy, dx) in enumerate(_OFFSETS):
            for n in range(128):
                k = n + shift + dy
                if 0 <= k < 128:
                    V[dx][k, n] += -float(2 ** j)
        for dx, tag in ((-1, "l"), (0, "c"), (1, "r")):
            out[f"v{tag}{name}"] = np.ascontiguousarray(
                V[dx].astype(ml_dtypes.bfloat16)
            )

    # cross-partition count weights: row k weighted n_k / (H*W); all 128
    # output columns identical -> the matmul broadcasts the totals.
    counts = np.zeros((128,), np.float64)
    for t in range(NT):
        n_out = TO if t < 8 else TAIL
        counts[0:n_out] += W
    wcnt = np.tile((counts / float(H * W)).astype(np.float32)[:, None], (1, 128))
    out["wcnt"] = np.ascontiguousarray(wcnt, dtype=np.float32)
    return out


def _mm_plan(lo_on):
    """Full-tile matmul plan, hi-source passes first (lo arrives later from
    the slower engine), then lo passes.  Entries:
    (vname, use_lo, h, in_c0, in_c1, out_c0, out_c1)."""
    def cols(vname, h):
        c0 = 512 * h
        if vname == "vc":
            return (c0, c0 + 512, 0, 512)
        if vname == "vl":
            return (0, 511, 1, 512) if h == 0 else (511, 1023, 0, 512)
        return (1, 513, 0, 512) if h == 0 else (513, 1024, 0, 511)

    plan = []
    for src_lo in (False, True):
        names = ("vc", "vl", "vr") if not src_lo else tuple(
            v for v in ("vc", "vl", "vr") if v in lo_on
        )
        for vname in names:
            for h in (0, 1):
                plan.append((vname, src_lo, h) + cols(vname, h))
    return plan


def build_nc(mode="hilo", lo_passes=("vc", "vl")):
    lo_on = lo_passes if mode == "hilo" else ()
    use_lo = len(lo_on) > 0
    nc = Bacc()
    x_in = nc.declare_dram_parameter("x", [B_PER_CORE, 1, H, W], F32, isOutput=False)
    out_ext = nc.declare_dram_parameter("out", [B_PER_CORE, 1, H, W], F32, isOutput=True)
    w_names = ["vla", "vca", "vra", "vlb", "vcb", "vrb"]
    w_dram = {
        n: nc.declare_dram_parameter(n, [128, 128], BF16, isOutput=False)
        for n in w_names
    }
    wcnt_d = nc.declare_dram_parameter("wcnt", [128, 128], F32, isOutput=False)

    with tile.TileContext(nc) as tc:
        with (
            tc.tile_pool(name="singles", bufs=1) as singles,
            tc.tile_pool(name="xp", bufs=8) as xp,
            tc.tile_pool(name="hp", bufs=6) as hp,
            tc.tile_pool(name="lp", bufs=6) as lp,
            tc.tile_pool(name="zp", bufs=2) as zp,
            tc.tile_pool(name="stat", bufs=2) as stat,
            tc.tile_pool(name="sm", bufs=4) as sm,
            tc.tile_pool(name="psp", bufs=3, space="PSUM") as psp,
            tc.tile_pool(name="pss", bufs=1, space="PSUM") as pss,
        ):
            prefetched = {}
            for pf_t in (0, 1, 2):
                in_a = max(TO * pf_t - 1, 0)
                in_b = min(TO * pf_t + TO + 1, H)
                xt = xp.tile([128, W], F32, tag="xt", name=f"xt_pf{pf_t}")
                nc.sync.dma_start(
                    out=xt[0:in_b - in_a, :], in_=x_in[0, 0, in_a:in_b, :]
                )
                prefetched[(0, pf_t)] = xt
            sb_v = {}
            for n in w_names:
                t_ = singles.tile([128, 128], BF16, tag=n)
                nc.sync.dma_start(out=t_, in_=w_dram[n][:, :])
                sb_v[n] = t_
            sb_wcnt = singles.tile([128, 128], F32, tag="wcnt")
            nc.sync.dma_start(out=sb_wcnt, in_=wcnt_d[:, :])
            sb_eps = singles.tile([128, 1], F32, tag="eps")
            nc.vector.memset(sb_eps, EPS_P)

            def emit_tile(s, t, z_big, stats):
                if t == 0:
                    nc.gpsimd.memset(stats[:], 0.0)
                n_out = TO if t < 8 else TAIL
                in_a = max(TO * t - 1, 0)       # first input row loaded
                in_b = min(TO * t + TO + 1, H)  # one past last input row
                rows = in_b - in_a              # 127/128, or 17 on the tail
                K = rows                        # contraction depth
                band = "a" if t == 0 else "b"

                if (s, t) in prefetched:
                    xt = prefetched.pop((s, t))
                else:
                    xt = xp.tile([128, W], F32, tag="xt")
                    nc.sync.dma_start(
                        out=xt[0:rows, :], in_=x_in[s, 0, in_a:in_b, :]
                    )
                hi = hp.tile([128, W], BF16, tag="hi")
                nc.scalar.copy(out=hi[0:rows, :], in_=xt[0:rows, :])
                lo = None
                if use_lo:
                    lo = lp.tile([128, W], BF16, tag="lo")
                    eng = nc.gpsimd if t % 2 == 0 else nc.vector
                    eng.tensor_sub(
                        lo[0:rows, :], xt[0:rows, :], hi[0:rows, :]
                    )

                ps = psp.tile([128, 2, 512], F32, tag="ps")
                plan = _mm_plan(lo_on)
                first_h = {0: True, 1: True}
                last_i = {h: max(i for i, p in enumerate(plan) if p[2] == h)
                          for h in (0, 1)}
                for i, (vname, src_lo, h, a, b, oa, ob) in enumerate(plan):
                    src = lo if src_lo else hi
                    nc.tensor.matmul(
                        ps[:, h, oa:ob],
                        lhsT=sb_v[vname + band][0:K, :],
                        rhs=src[0:K, a:b],
                        start=first_h[h],
                        stop=(i == last_i[h]),
                        skip_group_check=True,
                    )
                    first_h[h] = False

                nc.scalar.copy(
                    out=z_big[0:n_out, t, :].rearrange("p (g f) -> p g f", f=512),
                    in_=ps[0:n_out, :, :],
                )
                for g in (0, 1):
                    nc.vector.bn_stats(
                        out=stats[0:n_out, t, g, :],
                        in_=ps[0:n_out, g, :],
                    )

            def finalize_chunks(s, z_big, stats):
                box = {}

                def c1():
                    mv = box["mv"] = sm.tile([128, 2], F32, tag="mv", name="mv")
                    nc.vector.memset(mv, 0.0)
                    nc.vector.bn_aggr(out=mv[0:TO, :], in_=stats[0:TO, :, :, :])
                    msq = sm.tile([128, 1], F32, tag="msq")
                    nc.vector.tensor_mul(msq, mv[:, 0:1], mv[:, 0:1])
                    nc.vector.tensor_add(mv[:, 1:2], mv[:, 1:2], msq)  # E2

                def c2():
                    tot_ps = pss.tile([128, 2], F32, tag="totps")
                    nc.tensor.matmul(
                        tot_ps[:, :], lhsT=sb_wcnt[:, :], rhs=box["mv"][:, :],
                        start=True, stop=True,
                    )
                    tot = box["tot"] = sm.tile([128, 2], F32, tag="tot", name="tot")
                    nc.scalar.copy(out=tot, in_=tot_ps)

                def c3():
                    tot = box["tot"]
                    m2 = sm.tile([128, 1], F32, tag="m2")
                    nc.vector.tensor_mul(m2, tot[:, 0:1], tot[:, 0:1])
                    var = sm.tile([128, 1], F32, tag="var")
                    nc.vector.tensor_sub(var, tot[:, 1:2], m2)
                    sd = box["sd"] = sm.tile([128, 1], F32, tag="sd", name="sd")
                    nc.scalar.activation(
                        out=sd, in_=var, func=AF.Sqrt, bias=sb_eps, scale=1.0
                    )

                def c4():
                    inv = box["inv"] = sm.tile([128, 1], F32, tag="inv", name="inv")
                    nc.vector.reciprocal(inv, box["sd"])
                    nbias = box["nb"] = sm.tile([128, 1], F32, tag="nb", name="nb")
                    nc.vector.tensor_scalar(
                        out=nbias, in0=inv, scalar1=box["tot"][:, 0:1],
                        scalar2=-1.0, op0=ALU.mult, op1=ALU.mult,
                    )

                def norm_store(t0, t1):
                    def c():
                        nc.vector.tensor_scalar(
                            out=z_big[0:TO, t0:t1, :], in0=z_big[0:TO, t0:t1, :],
                            scalar1=box["inv"][0:TO, :],
                            scalar2=box["nb"][0:TO, :],
                            op0=ALU.mult, op1=ALU.add,
                        )
                        # output row 126t+n <-> (n, t) of z_big
                        nc.scalar.dma_start(
                            out=out_ext[s, 0, TO * t0:TO * t1, :].rearrange(
                                "(t n) w -> n t w", n=TO
                            ),
                            in_=z_big[0:TO, t0:t1, :],
                        )
                    return c

                def c_tail():
                    nc.vector.tensor_scalar(
                        out=z_big[0:TAIL, 8, :], in0=z_big[0:TAIL, 8, :],
                        scalar1=box["inv"][0:TAIL, :],
                        scalar2=box["nb"][0:TAIL, :],
                        op0=ALU.mult, op1=ALU.add,
                    )
                    nc.scalar.dma_start(
                        out=out_ext[s, 0, 8 * TO:H, :], in_=z_big[0:TAIL, 8, :]
                    )

                return [c1, c2, c3, c4,
                        norm_store(0, 4), norm_store(4, 8), c_tail]

            pending = []
            for s in range(B_PER_CORE):
                z_big = zp.tile([128, NT, W], F32, tag="z")
                stats = stat.tile([128, NT, 2, 6], F32, tag="stats")
                for t in range(NT):
                    emit_tile(s, t, z_big, stats)
                    if pending:
                        pending.pop(0)()
                while pending:
                    pending.pop(0)()
                pending = finalize_chunks(s, z_big, stats)
            while pending:
                pending.pop(0)()
    nc.finalize()
    return nc


_NC_CACHE = {}


def _get_nc(mode, lo_passes):
    key = (mode, tuple(lo_passes))
    if key not in _NC_CACHE:
        _NC_CACHE[key] = build_nc(mode, lo_passes)
    return _NC_CACHE[key]


def run(x, trace=False, mode="hilo", lo_passes=("vc", "vl"), tmpdir=None):
    x = np.ascontiguousarray(np.asarray(x), dtype=np.float32)
    assert x.shape == (N_CORES * B_PER_CORE, 1, H, W), x.shape
    weights = _build_host_weights()
    in_maps = []
    for c in range(N_CORES):
        m = {"x": x[c * B_PER_CORE:(c + 1) * B_PER_CORE]}
        m.update(weights)
        in_maps.append(m)
    nc = _get_nc(mode, lo_passes)
    res = run_bass_kernel_spmd(
        nc, in_maps, list(range(N_CORES)), trace=trace, tmpdir=tmpdir
    )
    out = np.concatenate([res.results[c]["out"] for c in range(N_CORES)], axis=0)
    return out, res


def kernel(x):
    out, _ = run(x, trace=False)
    return out
